# revision 10
# baseline (speedup 1.0000x reference)
"""Trainium2 Bass kernel for nn_BiLSTM_3410204033194.

The reference computes a 3-layer bidirectional LSTM over (T=1024, B=512,
IN=2) and then applies the final FC to out[:, -1, :] — the LAST BATCH
ELEMENT only.  LSTM batch elements are independent, so the full output
(T, 4) depends only on batch index 511.  We therefore run the whole
3-layer bidirectional recurrence for that single sequence on device
(data-parallel sharding degenerates to a single shard; all 8 cores run
the same SPMD program and we read core 0's output).

Device mapping (per scan step, both directions fused):
  - gate pre-activations pre[t] = W_ih@x[t] + b are computed by a bulk
    GEMM phase per layer, then injected into each step's PSUM
    accumulation group via an identity-stationary matmul (off the
    recurrent critical path).
  - recurrent matvec W_hh @ h[t-1] accumulates into the same PSUM tile
    (one small matmul per direction).
  - gates live in a quad layout (i@p0, f@p32, o@p64, g@p96) so that a
    single sigmoid instruction covers i,f,o and one tanh covers g
    (SBUF operand partition starts must be in {0,32,64,96}).
  - c update on the vector engine (3 tensor_tensor ops), tanh(c) on the
    scalar engine, h = sigmoid(o)*tanh(c) written straight into the
    per-direction sequence tiles that feed the next layer / FC.
"""
import os
import sys

sys.path.insert(0, "/opt/trn_rl_repo")

import numpy as np
from contextlib import ExitStack

import concourse.bass as bass
import concourse.tile as tile
from concourse import mybir
from concourse.bass_utils import run_bass_kernel_spmd

F32 = mybir.dt.float32
AF = mybir.ActivationFunctionType
ALU = mybir.AluOpType

H = 20
# source gate order is PyTorch's (i, f, g, o); quad placement f->0, i->1,
# o->2, g->3 keeps the sigmoid gates (f, i, o) partition-contiguous AND
# aligns (f with c) and (i with tanh(g)) for a single fused multiply over
# the combined ctg tile (c at rows 0..19, tanh(g) at rows 32..51).
GATE_QUAD = (1, 0, 3, 2)
NCORES = 8


# ---------------------------------------------------------------- host prep
def _pack_whh(w):
    """w: (4H, H) -> lhsT (H, 128) with gate g at columns 32q..32q+19."""
    out = np.zeros((H, 128), np.float32)
    for g in range(4):
        q = GATE_QUAD[g]
        out[:, 32 * q:32 * q + H] = w[H * g:H * (g + 1), :].T
    return out


def _pack_wih_a(w, b):
    """w: (4H, nin<=20), b: (4H,) -> (33, 128); row 32 carries the bias."""
    nin = w.shape[1]
    out = np.zeros((33, 128), np.float32)
    for g in range(4):
        q = GATE_QUAD[g]
        out[0:nin, 32 * q:32 * q + H] = w[H * g:H * (g + 1), :].T
        out[32, 32 * q:32 * q + H] = b[H * g:H * (g + 1)]
    return out


def _pack_wih_b(w):
    """w: (4H, 20) -> (20, 128)."""
    out = np.zeros((H, 128), np.float32)
    for g in range(4):
        q = GATE_QUAD[g]
        out[0:H, 32 * q:32 * q + H] = w[H * g:H * (g + 1), :].T
    return out


def prep_inputs(x, w_ih0, w_hh0, b0, w_ih12, w_hh12, b12, fc_w, fc_b, t_len):
    arrs = {}
    X0 = np.zeros((33, t_len), np.float32)
    X0[0:2, :] = np.asarray(x[:t_len, -1, :], np.float32).T
    X0[32, :] = 1.0
    arrs["X0"] = X0
    arrs["ident"] = np.eye(128, dtype=np.float32)
    for d in range(2):
        arrs[f"whh_0_{d}"] = _pack_whh(np.asarray(w_hh0[d], np.float32))
        arrs[f"wia_0_{d}"] = _pack_wih_a(
            np.asarray(w_ih0[d], np.float32), np.asarray(b0[d], np.float32))
    for l in (1, 2):
        for d in range(2):
            wih = np.asarray(w_ih12[l - 1, d], np.float32)
            arrs[f"whh_{l}_{d}"] = _pack_whh(np.asarray(w_hh12[l - 1, d], np.float32))
            arrs[f"wia_{l}_{d}"] = _pack_wih_a(
                wih[:, 0:H], np.asarray(b12[l - 1, d], np.float32))
            arrs[f"wib_{l}_{d}"] = _pack_wih_b(wih[:, H:2 * H])
    fc_w = np.asarray(fc_w, np.float32)
    arrs["fc_f"] = np.ascontiguousarray(fc_w[:, 0:H].T)
    arrs["fc_bw"] = np.ascontiguousarray(fc_w[:, H:2 * H].T)
    arrs["fc_bias"] = np.asarray(fc_b, np.float32).reshape(1, 4)
    return arrs


def input_specs(t_len):
    specs = {"X0": (33, t_len), "ident": (128, 128),
             "fc_f": (H, 4), "fc_bw": (H, 4), "fc_bias": (1, 4)}
    for d in range(2):
        specs[f"whh_0_{d}"] = (H, 128)
        specs[f"wia_0_{d}"] = (33, 128)
    for l in (1, 2):
        for d in range(2):
            specs[f"whh_{l}_{d}"] = (H, 128)
            specs[f"wia_{l}_{d}"] = (33, 128)
            specs[f"wib_{l}_{d}"] = (H, 128)
    return specs


# ---------------------------------------------------------------- device IR
def emit(ctx: ExitStack, tc: tile.TileContext, ins: dict, y_out, t_len: int):
    """ins: dict name -> DRAM AP;  y_out: DRAM AP (4, t_len)."""
    nc = tc.nc
    CH = min(512, t_len)  # psum free-dim chunk for the bulk GEMM phases
    nch = t_len // CH

    wp = ctx.enter_context(tc.tile_pool(name="wp", bufs=1))
    prep = ctx.enter_context(tc.tile_pool(name="prep", bufs=2))
    gp = ctx.enter_context(tc.tile_pool(name="gp", bufs=3))
    sps = ctx.enter_context(tc.tile_pool(name="sps", bufs=4, space="PSUM"))
    pps = ctx.enter_context(tc.tile_pool(name="pps", bufs=2, space="PSUM"))
    fps = ctx.enter_context(tc.tile_pool(name="fps", bufs=1, space="PSUM"))

    # --- load weights / inputs
    w = {}
    for name, ap in ins.items():
        t = wp.tile(list(ap.shape), F32, tag=name)
        nc.sync.dma_start(t[:], ap[:])
        w[name] = t

    # --- persistent sequence tiles: rows 0..19 h, rows 20..31 zero, row 32 ones
    S = {}
    for l in range(3):
        for d in range(2):
            s = wp.tile([33, t_len], F32, tag=f"S_{l}_{d}")
            nc.vector.memset(s[:], 0.0)
            nc.vector.memset(s[32:33, :], 1.0)
            S[l, d] = s
    # ctg: rows 0..19 = c state, rows 32..51 = tanh(g); rows 20..31 stay 0
    ctg = wp.tile([52, 2], F32, tag="ctg_state")
    ones = wp.tile([1, t_len], F32, tag="ones")
    nc.vector.memset(ones[:], 1.0)

    ident = w["ident"]

    for l in range(3):
        # ---- bulk input GEMM: pre_d[:, t] = W_ih @ x_l[t] + b (quad layout)
        pre = []
        for d in range(2):
            p = prep.tile([128, t_len], F32, tag="pre")
            pre.append(p)
        for chunk in range(nch):
            sl = slice(chunk * CH, (chunk + 1) * CH)
            for d in range(2):
                ps = pps.tile([128, CH], F32, tag="preps")
                if l == 0:
                    nc.tensor.matmul(ps[:], w[f"wia_0_{d}"][:], w["X0"][:, sl],
                                     start=True, stop=True)
                else:
                    nc.tensor.matmul(ps[:], w[f"wia_{l}_{d}"][:],
                                     S[l - 1, 0][:, sl], start=True, stop=False)
                    nc.tensor.matmul(ps[:], w[f"wib_{l}_{d}"][:],
                                     S[l - 1, 1][0:H, sl], start=False, stop=True)
                nc.scalar.copy(pre[d][:, sl], ps[:])

        # ---- recurrent scan (fwd time s, bwd time t_len-1-s, fused)
        nc.vector.memset(ctg[:], 0.0)
        whf, whb = w[f"whh_{l}_0"], w[f"whh_{l}_1"]
        Sf, Sb = S[l, 0], S[l, 1]
        for s in range(t_len):
            tb = t_len - 1 - s
            first = s == 0
            ps = sps.tile([128, 2], F32, tag="sps")
            nc.tensor.matmul(ps[:, 0:1], ident[:], pre[0][:, s:s + 1],
                             start=True, stop=False)
            nc.tensor.matmul(ps[:, 1:2], ident[:], pre[1][:, tb:tb + 1],
                             start=False, stop=first)
            if not first:
                nc.tensor.matmul(ps[:, 0:1], whf[:], Sf[0:H, s - 1:s],
                                 start=False, stop=False)
                nc.tensor.matmul(ps[:, 1:2], whb[:], Sb[0:H, tb + 1:tb + 2],
                                 start=False, stop=True)
            sg = gp.tile([84, 2], F32, tag="sg")
            nc.scalar.activation(sg[:], ps[0:84, :], AF.Sigmoid)
            nc.scalar.activation(ctg[32:52, :], ps[96:116, :], AF.Tanh)
            # tensor_tensor inputs must share a start partition (walrus
            # verifier); outputs may differ.
            q1 = gp.tile([H, 2], F32, tag="q1")
            q2 = gp.tile([H, 2], F32, tag="q2")
            nc.vector.tensor_mul(q1[:], sg[0:H, :], ctg[0:H, :])      # sig(f)*c
            nc.vector.tensor_mul(q2[:], sg[32:52, :], ctg[32:52, :])  # sig(i)*tg
            nc.vector.tensor_add(ctg[0:H, :], q1[:], q2[:])
            # tanh(c) lands at partition base 64 to align with sig(o)
            tct = gp.tile([84, 2], F32, tag="tct")
            nc.scalar.activation(tct[64:84, :], ctg[0:H, :], AF.Tanh)
            nc.vector.tensor_mul(Sf[0:H, s:s + 1], sg[64:84, 0:1],
                                 tct[64:84, 0:1])
            nc.vector.tensor_mul(Sb[0:H, tb:tb + 1], sg[64:84, 1:2],
                                 tct[64:84, 1:2])

    # ---- final FC: y = fc_w @ h_cat + fc_b  -> (4, t_len)
    ysb = wp.tile([4, t_len], F32, tag="ysb")
    for chunk in range(nch):
        sl = slice(chunk * CH, (chunk + 1) * CH)
        ps = fps.tile([4, CH], F32, tag="fcps")
        nc.tensor.matmul(ps[:], w["fc_f"][:], S[2, 0][0:H, sl],
                         start=True, stop=False)
        nc.tensor.matmul(ps[:], w["fc_bw"][:], S[2, 1][0:H, sl],
                         start=False, stop=False)
        nc.tensor.matmul(ps[:], w["fc_bias"][:], ones[:, sl],
                         start=False, stop=True)
        nc.scalar.copy(ysb[:, sl], ps[:])
    nc.sync.dma_start(y_out[:], ysb[:])


def _split_sem_waits(nc, cap=1):
    """The image's walrus supports at most `cap` sem waits per instruction
    ("Too many sync wait commands"); move extras onto preceding same-engine
    NoOps (engines are in-order, so an earlier wait is strictly stronger)."""
    for f in nc.m.functions:
        for bb in f.blocks:
            newlist = []
            changed = False
            for ins in bb.instructions:
                si = ins.sync_info
                if (si is not None and si.on_wait is not None
                        and len(si.on_wait) > cap
                        and not isinstance(ins, mybir.InstAllEngineBarrier)):
                    waits = list(si.on_wait)
                    extras, keep = waits[:-cap], waits[-cap:]
                    for j in range(0, len(extras), cap):
                        newlist.append(mybir.InstNoOp(
                            name=f"{ins.name}_xw{j}", engine=ins.engine,
                            ins=[], outs=[],
                            sync_info=mybir.SyncInfo(on_wait=extras[j:j + cap],
                                                     on_update=[])))
                    si.on_wait = keep
                    changed = True
                newlist.append(ins)
            if changed:
                bb.instructions = newlist


def build(t_len):
    nc = bass.Bass()
    aps = {}
    for name, shape in input_specs(t_len).items():
        aps[name] = nc.declare_dram_parameter(name, list(shape), F32,
                                              isOutput=False)
    y = nc.declare_dram_parameter("y_out", [4, t_len], F32, isOutput=True)
    with tile.TileContext(nc) as tc:
        with ExitStack() as ctx:
            emit(ctx, tc, aps, y, t_len)
    _split_sem_waits(nc)
    return nc


# ---------------------------------------------------------------- entrypoint
def run(inputs: dict, t_len=1024, trace=False, **kw):
    arrs = prep_inputs(**inputs, t_len=t_len)
    nc = build(t_len)
    in_maps = [arrs] * NCORES
    res = run_bass_kernel_spmd(nc, in_maps, list(range(NCORES)), trace=trace,
                               **kw)
    y = np.asarray(res.results[0]["y_out"])  # (4, t_len)
    return y.T.copy(), res


def kernel(**inputs) -> np.ndarray:
    y, _ = run(inputs, t_len=1024)
    return y.astype(np.float32)


if __name__ == "__main__":
    np.random.seed(1)
    T = int(os.environ.get("BASS_LSTM_T", "1024"))
    print(build(T))
